# revision 13
# baseline (speedup 1.0000x reference)
"""Trainium2 Bass kernel for nn_MeshLoss2D (chamfer distance between a point
cloud and a bilinearly-refined mesh).

Contract: kernel(vertices, pc) takes FULL inputs, returns the FULL (scalar)
output. Internally shards across 8 NeuronCores.

  chamfer = mean_p min_q d(p,q) + mean_q min_p d(p,q),
  d(p,q) = |a_p|^2 + |b_q|^2 - 2 a_p . b_q

Single-d design: the distance matrix is computed ONCE per core (queries = its
1024 pc rows x 2 batches, candidates = the full 9216-padded mesh) by K=13
compensated-bf16 matmuls (hi/lo split, see _fill_queries/_fill_cands).  Each
PSUM group is egressed once to fp16 SBUF (split ACT/DVE by a static
error-diffusion schedule; the Pool engine cannot read PSUM), and the fp16 copy is consumed twice:

  AB (pc->mesh rowmin): ONE tensor_scalar op per row-tile with op1=min
     accum_out - runs in the DVE 4x_2p fast mode (0.26 ns/col).
  BA (mesh->pc colmin): running elementwise fp16 TT-min into a per-batch
     [128, 9216] accumulator on DVE (the Pool engine has no tensor-tensor
     min on TRN2 hardware; it is idle in this kernel).

The BA accumulator (partition-axis still unreduced) is DMA'd out per batch;
the host does the cheap 128-way + cross-core min and the means.
"""

import sys

sys.path.insert(0, "/opt/trn_rl_repo")

import ml_dtypes
import numpy as np

import concourse.mybir as mybir
from concourse import bacc
from concourse.bass_utils import run_bass_kernel_spmd
from concourse.tile import TileContext

# ---- problem constants (hardcoded; kernel.py must be self-contained) ----
N_BATCH = 2
P = 8192                # point-cloud points per batch
Q = 95 * 95             # 9025 refined mesh points per batch
N_CORES = 8
KDIM = 13               # augmentation slots (hi/lo split product + both norms)

Q_PAD = 9216            # mesh candidates padded (= 18*512)
RPC = P // N_CORES      # 1024 pc-query rows per core per batch
RT = RPC // 128         # 8 row-tiles per batch
N_RT = N_BATCH * RT     # 16 row-tiles per core
CHUNK = 512             # matmul moving-operand width (ISA max)
EGROUP = 1024           # egress group width (2 psum banks)
PAD_D = 30000.0         # distance injected for pad candidates (< fp16 max)

# Egress engine split (by columns): ACT vs Pool. DVE is saturated by the
# AB tensor_scalar op + its BA share, so it takes no egress.
EGRESS_W = {"A": 0.86, "D": 0.14}
# BA column split between the parallel DVE and Pool min-chains (multiples
# of CHUNK; DVE fp16 TT runs 2x, Pool is ~2.7x slower per element).
BA_DVE_COLS = 9216      # all (Pool has no min ops on TRN2 HW)
BA_POOL_COLS = Q_PAD - BA_DVE_COLS

PSUM_BUFS = 4
E_BUFS = 3
ACC_BUFS = 4

_F32 = mybir.dt.float32
_F16 = mybir.dt.float16
_BF16 = mybir.dt.bfloat16
_BF16_NP = ml_dtypes.bfloat16


def _make_routes(n, weights):
    """Deterministic error-diffusion schedule hitting the weight ratios."""
    acc = dict.fromkeys(weights, 0.0)
    out = []
    for _ in range(n):
        for k in acc:
            acc[k] += weights[k]
        k = max(acc, key=lambda k: acc[k])
        acc[k] -= 1.0
        out.append(k)
    return out


def _build_nc(repeat=1, egress_w=None, ba_dve_cols=BA_DVE_COLS,
              egroup=EGROUP, psum_bufs=PSUM_BUFS, e_bufs=E_BUFS):
    egress_w = egress_w or EGRESS_W
    nc = bacc.Bacc("TRN2", target_bir_lowering=False)
    q_d = nc.dram_tensor("queries", [KDIM, N_RT * 128], _BF16, kind="ExternalInput")
    c_d = nc.dram_tensor("cands", [KDIM, N_BATCH * Q_PAD], _BF16, kind="ExternalInput")
    ab_d = nc.dram_tensor("abmins", [128, N_RT], _F32, kind="ExternalOutput")
    ba_d = nc.dram_tensor("bapart", [128, N_BATCH * Q_PAD], _F16, kind="ExternalOutput")

    # Egress unit widths per row-tile: 4 full groups + 1 half group.
    widths = [egroup] * (Q_PAD // egroup)
    if Q_PAD % egroup:
        widths.append(Q_PAD % egroup)
    n_units_total = repeat * N_RT * len(widths)
    routes = _make_routes(n_units_total, egress_w)

    with TileContext(nc) as tc:
        with (
            tc.tile_pool(name="const", bufs=1) as cpool,
            tc.tile_pool(name="psum", bufs=psum_bufs, space="PSUM") as ppool,
            tc.tile_pool(name="ebuf", bufs=e_bufs) as epool,
            tc.tile_pool(name="accb", bufs=ACC_BUFS) as apool,
        ):
            qt = cpool.tile([KDIM, N_RT * 128], _BF16)
            ct = cpool.tile([KDIM, N_BATCH * Q_PAD], _BF16)
            nc.sync.dma_start(out=qt[:], in_=q_d[:])
            for b in range(N_BATCH):
                nc.sync.dma_start(
                    out=ct[:, b * Q_PAD : (b + 1) * Q_PAD],
                    in_=c_d[:, b * Q_PAD : (b + 1) * Q_PAD],
                )
            abm = cpool.tile([128, N_RT], _F32)

            uidx = 0
            pending = None  # deferred consumers: (E, acc, b, t, rt_g, last_acc)

            def emit_consumers(p):
                E, acc, b, t, rt_g = p
                # AB: one 4x tensor_scalar with min-accum over the row-tile
                nc.vector.tensor_scalar(
                    out=E[:], in0=E[:], scalar1=0.0, scalar2=None,
                    op0=mybir.AluOpType.bypass, op1=mybir.AluOpType.min,
                    accum_out=abm[:, rt_g : rt_g + 1],
                )
                # BA: parallel DVE/Pool running-min chains (t=0 egressed
                # straight into acc, so it needs no init op)
                if t > 0:
                    if ba_dve_cols >= Q_PAD:
                        nc.vector.tensor_tensor(
                            acc[:], acc[:], E[:], op=mybir.AluOpType.min
                        )
                    else:
                        dv, pl = E[:, :ba_dve_cols], E[:, ba_dve_cols:]
                        av, al = acc[:, :ba_dve_cols], acc[:, ba_dve_cols:]
                        nc.vector.tensor_tensor(av, av, dv, op=mybir.AluOpType.min)
                        nc.gpsimd.tensor_tensor(al, al, pl, op=mybir.AluOpType.min)
                if t == RT - 1:
                    nc.sync.dma_start(
                        out=ba_d[:, b * Q_PAD : (b + 1) * Q_PAD], in_=acc[:]
                    )

            for _rep in range(repeat):
                for b in range(N_BATCH):
                    acc = apool.tile([128, Q_PAD], _F16, tag="acc")
                    for t in range(RT):
                        rt_g = b * RT + t
                        lhsT = qt[:, rt_g * 128 : (rt_g + 1) * 128]
                        # t=0 egresses straight into the BA accumulator: the
                        # first row-tile IS the initial running min
                        E = acc if t == 0 else epool.tile([128, Q_PAD], _F16, tag="E")
                        off = 0
                        for w in widths:
                            ps = ppool.tile([128, egroup], _F32)
                            for k in range(0, w, CHUNK):
                                nc.tensor.matmul(
                                    ps[:, k : k + CHUNK],
                                    lhsT,
                                    ct[:, b * Q_PAD + off + k : b * Q_PAD + off + k + CHUNK],
                                    start=True,
                                    stop=True,
                                )
                            if routes[uidx] == "A":
                                nc.scalar.copy(out=E[:, off : off + w], in_=ps[:, :w])
                            else:
                                # Pool cannot read PSUM on HW; DVE takes the
                                # non-ACT egress share
                                nc.vector.tensor_copy(
                                    out=E[:, off : off + w], in_=ps[:, :w]
                                )
                            uidx += 1
                            off += w
                        # consumers lag one row-tile so engine queues never
                        # head-of-line block on a not-yet-egressed E
                        if pending is not None:
                            emit_consumers(pending)
                        pending = (E, acc, b, t, rt_g)
                if pending is not None:
                    emit_consumers(pending)
                    pending = None
                nc.sync.dma_start(out=ab_d[:], in_=abm[:])
    nc.compile()
    return nc


_NC_CACHE = None


def _get_nc():
    global _NC_CACHE
    if _NC_CACHE is None:
        _NC_CACHE = _build_nc()
    return _NC_CACHE


class _Runner:
    """Persistent jitted shard_map runner (mirrors bass2jax.run_bass_via_pjrt
    but caches the jitted executable so repeated kernel() calls skip retrace)."""

    def __init__(self, nc, n_cores=N_CORES):
        import jax
        from jax.sharding import Mesh, PartitionSpec
        from jax.experimental.shard_map import shard_map
        from concourse import bass2jax

        bass2jax.install_neuronx_cc_hook()
        self._jax = jax
        self.n_cores = n_cores
        part_name = nc.partition_id_tensor.name if nc.partition_id_tensor else None
        in_names, out_names, out_avals, zero_shapes = [], [], [], []
        for alloc in nc.m.functions[0].allocations:
            if not isinstance(alloc, mybir.MemoryLocationSet):
                continue
            name = alloc.memorylocations[0].name
            if alloc.kind == "ExternalInput":
                if name != part_name:
                    in_names.append(name)
            elif alloc.kind == "ExternalOutput":
                out_names.append(name)
                shape = tuple(alloc.tensor_shape)
                dtype = mybir.dt.np(alloc.dtype)
                out_avals.append(jax.core.ShapedArray(shape, dtype))
                zero_shapes.append((shape, dtype))
        self.in_names, self.out_names = in_names, out_names
        self.out_shapes = [s for s, _ in zero_shapes]
        self.zero_shapes = zero_shapes
        n_params = len(in_names)
        all_names = in_names + out_names
        if part_name is not None:
            all_names = all_names + [part_name]

        def _body(*args):
            operands = list(args)
            if part_name is not None:
                operands.append(bass2jax.partition_id_tensor())
            return tuple(
                bass2jax._bass_exec_p.bind(
                    *operands,
                    out_avals=tuple(out_avals),
                    in_names=tuple(all_names),
                    out_names=tuple(out_names),
                    lowering_input_output_aliases=(),
                    sim_require_finite=True,
                    sim_require_nnan=True,
                    nc=nc,
                )
            )

        devices = jax.devices()[:n_cores]
        mesh = Mesh(np.asarray(devices), ("core",))
        n_out = len(out_names)
        self._fn = jax.jit(
            shard_map(
                _body,
                mesh=mesh,
                in_specs=(PartitionSpec("core"),) * (n_params + n_out),
                out_specs=(PartitionSpec("core"),) * n_out,
                check_rep=False,
            ),
            donate_argnums=tuple(range(n_params, n_params + n_out)),
            keep_unused=True,
        )

    def __call__(self, in_maps):
        concat_in = [
            np.concatenate([np.asarray(m[name]) for m in in_maps], axis=0)
            for name in self.in_names
        ]
        zeros = [
            np.zeros((self.n_cores * s[0], *s[1:]), d) for s, d in self.zero_shapes
        ]
        outs = self._fn(*concat_in, *zeros)
        self._jax.block_until_ready(outs)
        results = []
        for c in range(self.n_cores):
            results.append(
                {
                    name: np.asarray(outs[i]).reshape(
                        self.n_cores, *self.out_shapes[i]
                    )[c]
                    for i, name in enumerate(self.out_names)
                }
            )
        return results


_RUNNER_CACHE = None


def _get_runner():
    global _RUNNER_CACHE
    if _RUNNER_CACHE is None:
        _RUNNER_CACHE = _Runner(_get_nc())
    return _RUNNER_CACHE


def _upsample_last(x):
    """[..., W] -> [..., 2W-1] midpoint refinement (align_corners=True)."""
    mid = np.float32(0.5) * (x[..., :-1] + x[..., 1:])
    w = x.shape[-1]
    out = np.zeros(x.shape[:-1] + (2 * w - 1,), x.dtype)
    out[..., 0::2] = x
    out[..., 1::2] = mid
    return out


def _split(x):
    """f32 -> (hi, lo) bf16 pair with hi + lo ~= x."""
    h32 = x.astype(_BF16_NP).astype(np.float32)
    lo = (x - h32).astype(_BF16_NP)
    return h32.astype(_BF16_NP), lo


def _fill_queries(dst, pts, n2):
    """dst: [KDIM, n] bf16; pts: [n, 3] f32 queries; n2: [n] query norms."""
    h, l = _split(pts.T)                 # [3, n] each
    dst[0:3] = h
    dst[3:6] = h
    dst[6:9] = l
    dst[9] = _BF16_NP(1.0)
    dst[10] = _BF16_NP(1.0)
    n2h, n2l = _split(n2)
    dst[11] = n2h
    dst[12] = n2l


def _fill_cands(dst, pts, n2):
    """dst: [KDIM, n] bf16; pts: [n, 3] f32 candidates; n2: [n] cand norms."""
    h, l = _split(-2.0 * pts.T)          # exact *(-2) before split
    dst[0:3] = h
    dst[3:6] = l
    dst[6:9] = h
    n2h, n2l = _split(n2)
    dst[9] = n2h
    dst[10] = n2l
    dst[11] = _BF16_NP(1.0)
    dst[12] = _BF16_NP(1.0)


def _prep_inputs(vertices, pc):
    """Host prep: mesh refinement + augmented query/candidate matrices."""
    v = np.asarray(vertices, dtype=np.float32)
    a = np.asarray(pc, dtype=np.float32)                     # [n, P, 3]
    v = _upsample_last(v)                                    # refine W
    v = _upsample_last(v.swapaxes(-1, -2)).swapaxes(-1, -2)  # refine H
    top = v.reshape(N_BATCH, 3, -1).transpose(0, 2, 1)       # [n, Q, 3]

    a2 = np.sum(a * a, axis=-1)        # [n, P]
    b2 = np.sum(top * top, axis=-1)    # [n, Q]

    queries = [
        np.empty((KDIM, N_RT * 128), dtype=_BF16_NP) for _ in range(N_CORES)
    ]
    cands = np.zeros((KDIM, N_BATCH * Q_PAD), dtype=_BF16_NP)
    for b in range(N_BATCH):
        for c in range(N_CORES):
            sl = slice(c * RPC, (c + 1) * RPC)
            _fill_queries(
                queries[c][:, b * RPC : (b + 1) * RPC], a[b, sl], a2[b, sl]
            )
        dst = cands[:, b * Q_PAD : b * Q_PAD + Q]
        _fill_cands(dst, top[b], b2[b])
        # pad candidates: zero coords, huge norm -> never the min
        cands[9, b * Q_PAD + Q : (b + 1) * Q_PAD] = _BF16_NP(PAD_D)
        cands[11, b * Q_PAD + Q : (b + 1) * Q_PAD] = _BF16_NP(1.0)
        cands[12, b * Q_PAD + Q : (b + 1) * Q_PAD] = _BF16_NP(1.0)
    return queries, cands


def _combine(results):
    """Host combine: ab rowmins mean + ba partial accs (min over cores x
    partition rows, drop pads) mean."""
    ab_sum = 0.0
    ba = np.full((N_BATCH, Q_PAD), np.inf, dtype=np.float32)
    for c in range(N_CORES):
        ab_sum += np.sum(results[c]["abmins"].astype(np.float64))
        bp = results[c]["bapart"].astype(np.float32).reshape(128, N_BATCH, Q_PAD)
        np.minimum(ba, bp.min(axis=0), out=ba)
    ab_mean = ab_sum / (N_BATCH * P)
    ba_mean = float(np.mean(ba[:, :Q]))
    return np.float32(ab_mean + ba_mean)


def kernel(vertices, pc):
    queries, cands = _prep_inputs(vertices, pc)
    in_maps = [{"queries": queries[c], "cands": cands} for c in range(N_CORES)]
    try:
        results = _get_runner()(in_maps)
    except Exception:
        try:
            results = _get_runner()(in_maps)  # retry once (transient NRT errors)
        except Exception:
            # fallback: reference SPMD path (slower per call, same program)
            res = run_bass_kernel_spmd(
                _get_nc(), in_maps, core_ids=list(range(N_CORES))
            )
            results = res.results
    return np.asarray(_combine(results), dtype=np.float32)
